# revision 18
# baseline (speedup 1.0000x reference)
"""Contextual loss (cosine distance, 'regular') on 8 Trainium2 cores.

Math (N=1, C=256, S=96*96=9216):
  mean_T = gt.mean over spatial; I/T centered by mean_T, L2-normalized along C.
  cos[i,j] = Iv[:,i] . Tv[:,j]                       (S x S, via matmul over C)
  dist = clip((1-cos)/2, 0); rel = dist/(rowmin+eps); w = exp((1-rel)/0.5)
  cs = w / rowsum(w); loss = -log(mean_j max_i cs[i,j])

Key simplification (verified: clip never triggers for this data since
max cos ~= 0.52): with m_i = max(0,(1-rowmax_i)/2) and s_i = 1/(m_i+eps),
  cs[i,j] = exp(s_i*cos[i,j] - s_i) / Z_i,   Z_i = sum_j exp(s_i*cos[i,j] - s_i)
(the e^2 factor of the reference cancels in the normalization).

Sharding: rows i split 8 ways (1152 rows/core); each core owns its rows'
min/exp/sum completely and emits colmax[128, S] = max over its 9 row-blocks.
Host finishes with max over (8 cores x 128 partitions), mean, -log.

Per 128-row block (flash style, nothing S x S ever hits DRAM), software-
pipelined 3 deep so PE/ACT/DVE all stay busy across blocks:
  sweep1(b): matmul (fp16 in, fp32 PSUM) -> PSUM evac fp32->fp16 (ScalarE
             mostly; it is 1x on DVE anyway) -> row-max via fp16 TT-max
             chain (2x DVE) -> s = 1/(relu((1-rowmax)/2)+eps)
  sweep2(b): Exp activation with per-partition scale=s bias=-s, fused
             row-sum via accum_out -> w fp16, Z fp32; invZ = 1/Z
  sweep3(b): cs = w*invZ in place (4x tensor_scalar), column-max
             accumulate (2x tensor_tensor max)
"""

import numpy as np

C = 256
S = 96 * 96            # 9216
N_CORES = 8
ROWS_PER_CORE = S // N_CORES   # 1152
BLOCKS = ROWS_PER_CORE // 128  # 9
JT = 1024                      # psum tile free size (2 banks)
NJT = S // JT                  # 9 j-tiles per block
MM_FREE = 512                  # matmul moving free size (1 bank)
EPS_REL = 1e-5
HALF = S // 2

_compiled = None


def _build(repeat: int = 1):
    import concourse.bass as bass
    import concourse.tile as tile
    from concourse import bacc, mybir

    f16 = mybir.dt.float16
    f32 = mybir.dt.float32

    nc = bacc.Bacc("TRN2", target_bir_lowering=False, debug=False,
                   num_devices=N_CORES)
    iv_d = nc.dram_tensor("iv", [C, ROWS_PER_CORE], f16, kind="ExternalInput")
    tv_d = nc.dram_tensor("tv", [C, S], f16, kind="ExternalInput")
    out_d = nc.dram_tensor("colmax", [128, S], f16, kind="ExternalOutput")

    with tile.TileContext(nc) as tc:
        with (
            tc.tile_pool(name="persist", bufs=1) as persist,
            tc.tile_pool(name="blocks", bufs=3) as blk,
            tc.tile_pool(name="stats", bufs=3) as st,
            tc.tile_pool(name="psum", bufs=4, space="PSUM") as pp,
        ):
            iv_sb = [persist.tile([128, ROWS_PER_CORE], f16, tag=f"iv{k}",
                                  name=f"iv_sb{k}") for k in range(2)]
            tv_sb = [persist.tile([128, S], f16, tag=f"tv{k}",
                                  name=f"tv_sb{k}") for k in range(2)]
            for k in range(2):
                nc.sync.dma_start(out=iv_sb[k][:], in_=iv_d[k * 128:(k + 1) * 128, :])
            for j in range(0, S, 2304):
                for k in range(2):
                    nc.sync.dma_start(out=tv_sb[k][:, j:j + 2304],
                                      in_=tv_d[k * 128:(k + 1) * 128, j:j + 2304])

            colmax = persist.tile([128, S], f16, tag="colmax")
            nc.gpsimd.memset(colmax[:], 0.0)

            def sweep1(b):
                """MMs + evac + rowmax chain + s stats. Returns tiles."""
                bsl = slice(b * 128, (b + 1) * 128)
                cos = blk.tile([128, S], f16, tag="cos", name=f"cos{b}")
                racc = st.tile([128, JT], f16, tag="racc", name=f"racc{b}")
                for ti in range(NJT):
                    joff = ti * JT
                    ps = pp.tile([128, JT], f32, tag="ps", name=f"ps{b}_{ti}")
                    for k in range(2):
                        for q in range(JT // MM_FREE):
                            nc.tensor.matmul(
                                ps[:, q * MM_FREE:(q + 1) * MM_FREE],
                                iv_sb[k][:, bsl],
                                tv_sb[k][:, joff + q * MM_FREE:
                                          joff + (q + 1) * MM_FREE],
                                start=(k == 0), stop=(k == 1),
                            )
                    # PSUM evac fp32->fp16; last two tiles on DVE for balance
                    if ti < NJT - 2:
                        nc.scalar.copy(cos[:, joff:joff + JT], ps[:])
                    else:
                        nc.vector.tensor_copy(cos[:, joff:joff + JT], ps[:])
                    # row-max accumulation chain (fp16 TT-max, 2x mode)
                    if ti == 0:
                        nc.vector.tensor_copy(racc[:], cos[:, 0:JT])
                    else:
                        nc.vector.tensor_max(racc[:], racc[:],
                                             cos[:, joff:joff + JT])
                rowmax = st.tile([128, 1], f32, tag="rowmax", name=f"rmx{b}")
                nc.vector.reduce_max(rowmax[:], racc[:],
                                     axis=mybir.AxisListType.X)
                # me = max(0,(1-rowmax)/2) + eps  (clamp matters only if
                # rowmax > 1); s = 1/me; neg_s = -s
                me = st.tile([128, 1], f32, tag="me", name=f"me{b}")
                nc.vector.tensor_scalar(me[:], rowmax[:], -0.5, 0.5 + EPS_REL,
                                        op0=mybir.AluOpType.mult,
                                        op1=mybir.AluOpType.add)
                nc.vector.tensor_scalar_max(me[:], me[:], EPS_REL)
                s_t = st.tile([128, 1], f32, tag="s", name=f"s{b}")
                nc.vector.reciprocal(s_t[:], me[:])
                neg_s = st.tile([128, 1], f32, tag="neg_s", name=f"ns{b}")
                nc.vector.tensor_scalar_mul(neg_s[:], s_t[:], -1.0)
                return cos, s_t, neg_s

            def sweep2(b, cos, s_t, neg_s):
                """exp + fused row-sum; invZ."""
                w = blk.tile([128, S], f16, tag="w", name=f"w{b}")
                zpart = st.tile([128, 2], f32, tag="zpart", name=f"zp{b}")
                for h in range(2):
                    hsl = slice(h * HALF, (h + 1) * HALF)
                    nc.scalar.activation(w[:, hsl], cos[:, hsl],
                                         mybir.ActivationFunctionType.Exp,
                                         bias=neg_s[:], scale=s_t[:],
                                         accum_out=zpart[:, h:h + 1])
                z = st.tile([128, 1], f32, tag="z", name=f"z{b}")
                nc.vector.reduce_sum(z[:], zpart[:], axis=mybir.AxisListType.X)
                inv_z = st.tile([128, 1], f32, tag="inv_z", name=f"iz{b}")
                nc.vector.reciprocal(inv_z[:], z[:])
                return w, inv_z

            def sweep3(b, w, inv_z):
                """cs = w*invZ in place (Pool engine), colmax accumulate
                (2x TT-max on DVE). On the final block, DMA each finished
                half out immediately."""
                for h in range(2):
                    hsl = slice(h * HALF, (h + 1) * HALF)
                    nc.vector.tensor_scalar_mul(w[:, hsl], w[:, hsl], inv_z[:])
                    nc.vector.tensor_max(colmax[:, hsl], colmax[:, hsl],
                                         w[:, hsl])
                    if b == BLOCKS - 1:
                        nc.sync.dma_start(out=out_d[:, hsl], in_=colmax[:, hsl])

            for rep in range(repeat):
                s1 = {}
                s2 = {}
                for i in range(BLOCKS + 2):
                    # emission order = scheduling priority: older blocks'
                    # later sweeps first so ACT/DVE fill PE's matmul window
                    if i >= 1 and (i - 1) in s1:
                        s2[i - 1] = sweep2(i - 1, *s1.pop(i - 1))
                    if i >= 2 and (i - 2) in s2:
                        sweep3(i - 2, *s2.pop(i - 2))
                    if i < BLOCKS:
                        s1[i] = sweep1(i)

    nc.compile()
    return nc


def _get_compiled():
    global _compiled
    if _compiled is None:
        _compiled = _build()
    return _compiled


def _preprocess(images: np.ndarray, gt: np.ndarray):
    x = np.asarray(images, np.float32)[0].reshape(C, S)
    t = np.asarray(gt, np.float32)[0].reshape(C, S)
    mean_t = t.mean(axis=1, dtype=np.float32).astype(np.float32)
    i_c = x - mean_t[:, None]
    t_c = t - mean_t[:, None]
    i_n = np.sqrt((i_c * i_c).sum(axis=0, dtype=np.float32)).astype(np.float32)
    t_n = np.sqrt((t_c * t_c).sum(axis=0, dtype=np.float32)).astype(np.float32)
    iv = (i_c / np.maximum(i_n, 1e-12)).astype(np.float16)
    tv = (t_c / np.maximum(t_n, 1e-12)).astype(np.float16)
    return iv, tv


def kernel(images: np.ndarray, gt: np.ndarray) -> np.ndarray:
    from concourse.bass_utils import run_bass_kernel_spmd

    nc = _get_compiled()
    iv, tv = _preprocess(images, gt)
    in_maps = [
        {"iv": np.ascontiguousarray(iv[:, c * ROWS_PER_CORE:(c + 1) * ROWS_PER_CORE]),
         "tv": tv}
        for c in range(N_CORES)
    ]
    res = run_bass_kernel_spmd(nc, in_maps, list(range(N_CORES)))
    colmax = np.stack([res.results[c]["colmax"] for c in range(N_CORES)])
    cs_max = colmax.astype(np.float32).max(axis=(0, 1))       # [S]
    loss = -np.log(cs_max.mean(dtype=np.float32))
    return np.asarray(loss, dtype=np.float32)


# revision 23
# speedup vs baseline: 13098.5960x; 13098.5960x over previous
"""Contextual loss (cosine distance, 'regular') on 8 Trainium2 cores.

Math (N=1, C=256, S=96*96=9216):
  mean_T = gt.mean over spatial; I/T centered by mean_T, L2-normalized along C.
  cos[i,j] = Iv[:,i] . Tv[:,j]                       (S x S, via matmul over C)
  dist = clip((1-cos)/2, 0); rel = dist/(rowmin+eps); w = exp((1-rel)/0.5)
  cs = w / rowsum(w); loss = -log(mean_j max_i cs[i,j])

Key simplification (verified: clip never triggers for this data since
max cos ~= 0.52): with m_i = max(0,(1-rowmax_i)/2) and s_i = 1/(m_i+eps),
  cs[i,j] = exp(s_i*cos[i,j] - s_i) / Z_i,   Z_i = sum_j exp(s_i*cos[i,j] - s_i)
(the e^2 factor of the reference cancels in the normalization).

Sharding: rows i split 8 ways (1152 rows/core); each core owns its rows'
min/exp/sum completely and emits colmax[128, S] = max over its 9 row-blocks.
Host finishes with max over (8 cores x 128 partitions), mean, -log.

Per 128-row block (flash style, nothing S x S ever hits DRAM), software-
pipelined 3 deep so PE/ACT/DVE all stay busy across blocks:
  sweep1(b): matmul (fp16 in, fp32 PSUM) -> PSUM evac fp32->fp16 (ScalarE
             mostly; it is 1x on DVE anyway) -> row-max via fp16 TT-max
             chain (2x DVE) -> s = 1/(relu((1-rowmax)/2)+eps)
  sweep2(b): Exp activation with per-partition scale=s bias=-s, fused
             row-sum via accum_out -> w fp16, Z fp32; invZ = 1/Z
  sweep3(b): cs = w*invZ in place (4x tensor_scalar), column-max
             accumulate (2x tensor_tensor max)
"""

import numpy as np

C = 256
S = 96 * 96            # 9216
N_CORES = 8
ROWS_PER_CORE = S // N_CORES   # 1152
BLOCKS = ROWS_PER_CORE // 128  # 9
JT = 1024                      # psum tile free size (2 banks)
NJT = S // JT                  # 9 j-tiles per block
MM_FREE = 512                  # matmul moving free size (1 bank)
EPS_REL = 1e-5
HALF = S // 2

_compiled = None


def _build(repeat: int = 1, dve_evac: int = 3, mm_free: int = MM_FREE):
    import concourse.bass as bass
    import concourse.tile as tile
    from concourse import bacc, mybir

    f16 = mybir.dt.float16
    f32 = mybir.dt.float32

    nc = bacc.Bacc("TRN2", target_bir_lowering=False, debug=False,
                   num_devices=N_CORES)
    iv_d = nc.dram_tensor("iv", [C, ROWS_PER_CORE], f16, kind="ExternalInput")
    tv_d = nc.dram_tensor("tv", [C, S], f16, kind="ExternalInput")
    out_d = nc.dram_tensor("colmax", [128, S], f16, kind="ExternalOutput")

    with tile.TileContext(nc) as tc:
        with (
            tc.tile_pool(name="persist", bufs=1) as persist,
            tc.tile_pool(name="blocks", bufs=3) as blk,
            tc.tile_pool(name="stats", bufs=3) as st,
            tc.tile_pool(name="psum", bufs=4, space="PSUM") as pp,
        ):
            iv_sb = [persist.tile([128, ROWS_PER_CORE], f16, tag=f"iv{k}",
                                  name=f"iv_sb{k}") for k in range(2)]
            tv_sb = [persist.tile([128, S], f16, tag=f"tv{k}",
                                  name=f"tv_sb{k}") for k in range(2)]
            for k in range(2):
                nc.sync.dma_start(out=iv_sb[k][:], in_=iv_d[k * 128:(k + 1) * 128, :])
            for j in range(0, S, 2304):
                for k in range(2):
                    nc.sync.dma_start(out=tv_sb[k][:, j:j + 2304],
                                      in_=tv_d[k * 128:(k + 1) * 128, j:j + 2304])

            colmax = persist.tile([128, S], f16, tag="colmax")
            nc.gpsimd.memset(colmax[:], 0.0)

            def sweep1(b):
                """MMs + evac + rowmax chain + s stats. Returns tiles."""
                bsl = slice(b * 128, (b + 1) * 128)
                cos = blk.tile([128, S], f16, tag="cos", name=f"cos{b}")
                racc = st.tile([128, JT], f16, tag="racc", name=f"racc{b}")
                for ti in range(NJT):
                    joff = ti * JT
                    ps = pp.tile([128, JT], f32, tag="ps", name=f"ps{b}_{ti}")
                    for k in range(2):
                        for q in range(JT // mm_free):
                            nc.tensor.matmul(
                                ps[:, q * mm_free:(q + 1) * mm_free],
                                iv_sb[k][:, bsl],
                                tv_sb[k][:, joff + q * mm_free:
                                          joff + (q + 1) * mm_free],
                                start=(k == 0), stop=(k == 1),
                            )
                    # PSUM evac fp32->fp16; last dve_evac tiles on DVE
                    if ti < NJT - dve_evac:
                        nc.scalar.copy(cos[:, joff:joff + JT], ps[:])
                    else:
                        nc.vector.tensor_copy(cos[:, joff:joff + JT], ps[:])
                    # row-max accumulation chain (fp16 TT-max, 2x mode)
                    if ti == 0:
                        nc.vector.tensor_copy(racc[:], cos[:, 0:JT])
                    else:
                        nc.vector.tensor_max(racc[:], racc[:],
                                             cos[:, joff:joff + JT])
                rowmax = st.tile([128, 1], f32, tag="rowmax", name=f"rmx{b}")
                nc.vector.reduce_max(rowmax[:], racc[:],
                                     axis=mybir.AxisListType.X)
                # me = max(0,(1-rowmax)/2) + eps  (clamp matters only if
                # rowmax > 1); s = 1/me; neg_s = -s
                me = st.tile([128, 1], f32, tag="me", name=f"me{b}")
                nc.vector.tensor_scalar(me[:], rowmax[:], -0.5, 0.5 + EPS_REL,
                                        op0=mybir.AluOpType.mult,
                                        op1=mybir.AluOpType.add)
                nc.vector.tensor_scalar_max(me[:], me[:], EPS_REL)
                s_t = st.tile([128, 1], f32, tag="s", name=f"s{b}")
                nc.vector.reciprocal(s_t[:], me[:])
                neg_s = st.tile([128, 1], f32, tag="neg_s", name=f"ns{b}")
                nc.vector.tensor_scalar_mul(neg_s[:], s_t[:], -1.0)
                return cos, s_t, neg_s

            def sweep2(b, cos, s_t, neg_s):
                """exp + fused row-sum; invZ."""
                w = blk.tile([128, S], f16, tag="w", name=f"w{b}")
                zpart = st.tile([128, 2], f32, tag="zpart", name=f"zp{b}")
                for h in range(2):
                    hsl = slice(h * HALF, (h + 1) * HALF)
                    nc.scalar.activation(w[:, hsl], cos[:, hsl],
                                         mybir.ActivationFunctionType.Exp,
                                         bias=neg_s[:], scale=s_t[:],
                                         accum_out=zpart[:, h:h + 1])
                z = st.tile([128, 1], f32, tag="z", name=f"z{b}")
                nc.vector.reduce_sum(z[:], zpart[:], axis=mybir.AxisListType.X)
                inv_z = st.tile([128, 1], f32, tag="inv_z", name=f"iz{b}")
                nc.vector.reciprocal(inv_z[:], z[:])
                return w, inv_z

            def sweep3(b, w, inv_z):
                """cs = w*invZ in place (Pool engine), colmax accumulate
                (2x TT-max on DVE). On the final block, DMA each finished
                half out immediately."""
                for h in range(2):
                    hsl = slice(h * HALF, (h + 1) * HALF)
                    nc.vector.tensor_scalar_mul(w[:, hsl], w[:, hsl], inv_z[:])
                    nc.vector.tensor_max(colmax[:, hsl], colmax[:, hsl],
                                         w[:, hsl])
                    if b == BLOCKS - 1:
                        nc.sync.dma_start(out=out_d[:, hsl], in_=colmax[:, hsl])

            for rep in range(repeat):
                s1 = {}
                s2 = {}
                for i in range(BLOCKS + 2):
                    # emission order = scheduling priority: older blocks'
                    # later sweeps first so ACT/DVE fill PE's matmul window
                    if i >= 1 and (i - 1) in s1:
                        s2[i - 1] = sweep2(i - 1, *s1.pop(i - 1))
                    if i >= 2 and (i - 2) in s2:
                        sweep3(i - 2, *s2.pop(i - 2))
                    if i < BLOCKS:
                        s1[i] = sweep1(i)

    nc.compile()
    return nc


def _get_compiled():
    global _compiled
    if _compiled is None:
        _compiled = _build()
    return _compiled


def _preprocess(images: np.ndarray, gt: np.ndarray):
    x = np.asarray(images, np.float32)[0].reshape(C, S)
    t = np.asarray(gt, np.float32)[0].reshape(C, S)
    mean_t = t.mean(axis=1, dtype=np.float32).astype(np.float32)
    i_c = x - mean_t[:, None]
    t_c = t - mean_t[:, None]
    i_n = np.sqrt((i_c * i_c).sum(axis=0, dtype=np.float32)).astype(np.float32)
    t_n = np.sqrt((t_c * t_c).sum(axis=0, dtype=np.float32)).astype(np.float32)
    iv = (i_c / np.maximum(i_n, 1e-12)).astype(np.float16)
    tv = (t_c / np.maximum(t_n, 1e-12)).astype(np.float16)
    return iv, tv


def kernel(images: np.ndarray, gt: np.ndarray) -> np.ndarray:
    from concourse.bass_utils import run_bass_kernel_spmd

    nc = _get_compiled()
    iv, tv = _preprocess(images, gt)
    in_maps = [
        {"iv": np.ascontiguousarray(iv[:, c * ROWS_PER_CORE:(c + 1) * ROWS_PER_CORE]),
         "tv": tv}
        for c in range(N_CORES)
    ]
    res = run_bass_kernel_spmd(nc, in_maps, list(range(N_CORES)))
    colmax = np.stack([res.results[c]["colmax"] for c in range(N_CORES)])
    cs_max = colmax.astype(np.float32).max(axis=(0, 1))       # [S]
    loss = -np.log(cs_max.mean(dtype=np.float32))
    return np.asarray(loss, dtype=np.float32)


# revision 25
# speedup vs baseline: 15121.9927x; 1.1545x over previous
"""Contextual loss (cosine distance, 'regular') on 8 Trainium2 cores.

Math (N=1, C=256, S=96*96=9216):
  mean_T = gt.mean over spatial; I/T centered by mean_T, L2-normalized along C.
  cos[i,j] = Iv[:,i] . Tv[:,j]                       (S x S, via matmul over C)
  dist = clip((1-cos)/2, 0); rel = dist/(rowmin+eps); w = exp((1-rel)/0.5)
  cs = w / rowsum(w); loss = -log(mean_j max_i cs[i,j])

Key simplification (verified: clip never triggers for this data since
max cos ~= 0.52): with m_i = max(0,(1-rowmax_i)/2) and s_i = 1/(m_i+eps),
  cs[i,j] = exp(s_i*cos[i,j] - s_i) / Z_i,   Z_i = sum_j exp(s_i*cos[i,j] - s_i)
(the e^2 factor of the reference cancels in the normalization).

Sharding: rows i split 8 ways (1152 rows/core); each core owns its rows'
min/exp/sum completely and emits colmax[128, S] = max over its 9 row-blocks.
Host finishes with max over (8 cores x 128 partitions), mean, -log.

Per 128-row block (flash style, nothing S x S ever hits DRAM), software-
pipelined 3 deep so PE/ACT/DVE all stay busy across blocks:
  sweep1(b): matmul (fp16 in, fp32 PSUM) -> PSUM evac fp32->fp16 (ScalarE
             mostly; it is 1x on DVE anyway) -> row-max via fp16 TT-max
             chain (2x DVE) -> s = 1/(relu((1-rowmax)/2)+eps)
  sweep2(b): Exp activation with per-partition scale=s bias=-s, fused
             row-sum via accum_out -> w fp16, Z fp32; invZ = 1/Z
  sweep3(b): cs = w*invZ in place (4x tensor_scalar), column-max
             accumulate (2x tensor_tensor max)
"""

import numpy as np

C = 256
S = 96 * 96            # 9216
N_CORES = 8
ROWS_PER_CORE = S // N_CORES   # 1152
BLOCKS = ROWS_PER_CORE // 128  # 9
JT = 1024                      # psum tile free size (2 banks)
NJT = S // JT                  # 9 j-tiles per block
MM_FREE = 512                  # matmul moving free size (1 bank)
EPS_REL = 1e-5
HALF = S // 2

_compiled = None


def _build(repeat: int = 1, dve_evac: int = 2, mm_free: int = MM_FREE):
    import concourse.bass as bass
    import concourse.tile as tile
    from concourse import bacc, mybir

    f16 = mybir.dt.float16
    f32 = mybir.dt.float32

    nc = bacc.Bacc("TRN2", target_bir_lowering=False, debug=False,
                   num_devices=N_CORES)
    iv_d = nc.dram_tensor("iv", [C, ROWS_PER_CORE], f16, kind="ExternalInput")
    tv_d = nc.dram_tensor("tv", [C, S], f16, kind="ExternalInput")
    out_d = nc.dram_tensor("colmax", [128, S], f16, kind="ExternalOutput")

    with tile.TileContext(nc) as tc:
        with (
            tc.tile_pool(name="persist", bufs=1) as persist,
            tc.tile_pool(name="blocks", bufs=3) as blk,
            tc.tile_pool(name="stats", bufs=3) as st,
            tc.tile_pool(name="psum", bufs=4, space="PSUM") as pp,
        ):
            iv_sb = [persist.tile([128, ROWS_PER_CORE], f16, tag=f"iv{k}",
                                  name=f"iv_sb{k}") for k in range(2)]
            tv_sb = [persist.tile([128, S], f16, tag=f"tv{k}",
                                  name=f"tv_sb{k}") for k in range(2)]
            for k in range(2):
                nc.sync.dma_start(out=iv_sb[k][:], in_=iv_d[k * 128:(k + 1) * 128, :])
            # first j-tiles as small chunks so block 0's matmuls start early
            tv_chunks = [0, 1024, 2048, 4096, 6144, 8192, S]
            for j0, j1 in zip(tv_chunks[:-1], tv_chunks[1:]):
                for k in range(2):
                    nc.sync.dma_start(out=tv_sb[k][:, j0:j1],
                                      in_=tv_d[k * 128:(k + 1) * 128, j0:j1])

            colmax = persist.tile([128, S], f16, tag="colmax")
            nc.gpsimd.memset(colmax[:], 0.0)

            def sweep1(b):
                """MMs + evac + rowmax chain + s stats. Returns tiles."""
                bsl = slice(b * 128, (b + 1) * 128)
                cos = blk.tile([128, S], f16, tag="cos", name=f"cos{b}")
                racc = st.tile([128, JT], f16, tag="racc", name=f"racc{b}")
                for ti in range(NJT):
                    joff = ti * JT
                    ps = pp.tile([128, JT], f32, tag="ps", name=f"ps{b}_{ti}")
                    for k in range(2):
                        for q in range(JT // mm_free):
                            nc.tensor.matmul(
                                ps[:, q * mm_free:(q + 1) * mm_free],
                                iv_sb[k][:, bsl],
                                tv_sb[k][:, joff + q * mm_free:
                                          joff + (q + 1) * mm_free],
                                start=(k == 0), stop=(k == 1),
                            )
                    # PSUM evac fp32->fp16; last dve_evac tiles on DVE
                    if ti < NJT - dve_evac:
                        nc.scalar.copy(cos[:, joff:joff + JT], ps[:])
                    else:
                        nc.vector.tensor_copy(cos[:, joff:joff + JT], ps[:])
                    # row-max accumulation chain (fp16 TT-max, 2x mode)
                    if ti == 0:
                        nc.vector.tensor_copy(racc[:], cos[:, 0:JT])
                    else:
                        nc.vector.tensor_max(racc[:], racc[:],
                                             cos[:, joff:joff + JT])
                rowmax = st.tile([128, 1], f32, tag="rowmax", name=f"rmx{b}")
                nc.vector.reduce_max(rowmax[:], racc[:],
                                     axis=mybir.AxisListType.X)
                # me = max(0,(1-rowmax)/2) + eps  (clamp matters only if
                # rowmax > 1); s = 1/me; neg_s = -s
                me = st.tile([128, 1], f32, tag="me", name=f"me{b}")
                nc.vector.tensor_scalar(me[:], rowmax[:], -0.5, 0.5 + EPS_REL,
                                        op0=mybir.AluOpType.mult,
                                        op1=mybir.AluOpType.add)
                nc.vector.tensor_scalar_max(me[:], me[:], EPS_REL)
                s_t = st.tile([128, 1], f32, tag="s", name=f"s{b}")
                nc.vector.reciprocal(s_t[:], me[:])
                neg_s = st.tile([128, 1], f32, tag="neg_s", name=f"ns{b}")
                nc.vector.tensor_scalar_mul(neg_s[:], s_t[:], -1.0)
                return cos, s_t, neg_s

            def sweep2(b, cos, s_t, neg_s):
                """exp + fused row-sum; invZ."""
                w = blk.tile([128, S], f16, tag="w", name=f"w{b}")
                zpart = st.tile([128, 2], f32, tag="zpart", name=f"zp{b}")
                for h in range(2):
                    hsl = slice(h * HALF, (h + 1) * HALF)
                    nc.scalar.activation(w[:, hsl], cos[:, hsl],
                                         mybir.ActivationFunctionType.Exp,
                                         bias=neg_s[:], scale=s_t[:],
                                         accum_out=zpart[:, h:h + 1])
                z = st.tile([128, 1], f32, tag="z", name=f"z{b}")
                nc.vector.reduce_sum(z[:], zpart[:], axis=mybir.AxisListType.X)
                inv_z = st.tile([128, 1], f32, tag="inv_z", name=f"iz{b}")
                nc.vector.reciprocal(inv_z[:], z[:])
                return w, inv_z

            def sweep3(b, w, inv_z):
                """cs = w*invZ in place (Pool engine), colmax accumulate
                (2x TT-max on DVE). On the final block, DMA each finished
                half out immediately."""
                for h in range(2):
                    hsl = slice(h * HALF, (h + 1) * HALF)
                    nc.vector.tensor_scalar_mul(w[:, hsl], w[:, hsl], inv_z[:])
                    nc.vector.tensor_max(colmax[:, hsl], colmax[:, hsl],
                                         w[:, hsl])
                    if b == BLOCKS - 1:
                        nc.sync.dma_start(out=out_d[:, hsl], in_=colmax[:, hsl])

            for rep in range(repeat):
                s1 = {}
                s2 = {}
                for i in range(BLOCKS + 2):
                    # emission order = scheduling priority: older blocks'
                    # later sweeps first so ACT/DVE fill PE's matmul window
                    if i >= 1 and (i - 1) in s1:
                        s2[i - 1] = sweep2(i - 1, *s1.pop(i - 1))
                    if i >= 2 and (i - 2) in s2:
                        sweep3(i - 2, *s2.pop(i - 2))
                    if i < BLOCKS:
                        s1[i] = sweep1(i)

    nc.compile()
    return nc


def _get_compiled():
    global _compiled
    if _compiled is None:
        _compiled = _build()
    return _compiled


def _preprocess(images: np.ndarray, gt: np.ndarray):
    x = np.asarray(images, np.float32)[0].reshape(C, S)
    t = np.asarray(gt, np.float32)[0].reshape(C, S)
    mean_t = t.mean(axis=1, dtype=np.float32).astype(np.float32)
    i_c = x - mean_t[:, None]
    t_c = t - mean_t[:, None]
    i_n = np.sqrt((i_c * i_c).sum(axis=0, dtype=np.float32)).astype(np.float32)
    t_n = np.sqrt((t_c * t_c).sum(axis=0, dtype=np.float32)).astype(np.float32)
    iv = (i_c / np.maximum(i_n, 1e-12)).astype(np.float16)
    tv = (t_c / np.maximum(t_n, 1e-12)).astype(np.float16)
    return iv, tv


def kernel(images: np.ndarray, gt: np.ndarray) -> np.ndarray:
    from concourse.bass_utils import run_bass_kernel_spmd

    nc = _get_compiled()
    iv, tv = _preprocess(images, gt)
    in_maps = [
        {"iv": np.ascontiguousarray(iv[:, c * ROWS_PER_CORE:(c + 1) * ROWS_PER_CORE]),
         "tv": tv}
        for c in range(N_CORES)
    ]
    res = run_bass_kernel_spmd(nc, in_maps, list(range(N_CORES)))
    colmax = np.stack([res.results[c]["colmax"] for c in range(N_CORES)])
    cs_max = colmax.astype(np.float32).max(axis=(0, 1))       # [S]
    loss = -np.log(cs_max.mean(dtype=np.float32))
    return np.asarray(loss, dtype=np.float32)
